# revision 7
# baseline (speedup 1.0000x reference)
"""GraphUNet forward on 8 TRN2 NeuronCores — raw Bass, multi-launch SPMD.

Sharding: 1D node partition. Seven launches:
  gcn(4096) -> level(4096,2048) -> level(2048,1024) -> level(1024,512)
  -> gcn(1024) -> gcn(2048) -> gcn(4096)
Host does dense-adjacency build, top-k, tiny [n,16] gemms, gathers, degree
vectors, and all diagonal/self-loop corrections (so no masks ship to device).

Level NEFF computes the augmented-pooled adjacency TRANSPOSED directly:
  CT block = B^T @ R^T-cols   (B = A'[:,keep_c], rhs = A'^T[:,keep_r])
so the GCN aggregation (contraction over columns of C) needs no on-device
transposes at all. 2x4 core grid over (rows, cols) of C.

Dtypes: adjacency entries are small exact integers -> fp8e4 (levels 0-1 use
DoubleRow fp8 matmul, 2x128 contraction/instr), fp16 where entries exceed 15
(level 2, whose C can reach 5e2..7e7 -> fp32 aggregation path). Feature
vectors ride [hi|lo] fp16 packs multiplied in ONE matmul pass (lhsT 32-wide);
PSUM accumulates fp32, so top-k scores are exact to ~1e-6.
"""
import os
import numpy as np
import ml_dtypes

import concourse.bass as bass
import concourse.mybir as mybir
from concourse.bass_utils import run_bass_kernel_spmd

N, E, F, D = 4096, 131072, 14, 16
NCORE = 8
KS = [2048, 1024, 512]
RG, CG = 2, 4  # level-core grid: RG row-blocks x CG col-blocks of C

f8 = ml_dtypes.float8_e4m3
f16 = np.float16
DT8 = mybir.dt.float8e4
DT16 = mybir.dt.float16
DT32 = mybir.dt.float32
DT32R = mybir.dt.float32r


def _can_trace():
    if os.environ.get("BASS_PROF") != "1":
        return False
    try:
        import antenv.axon_hooks  # noqa: F401
        return True
    except ImportError:
        return False


TRACE = _can_trace()
_tns = [0]
_cache = {}


def _run(nc, in_maps):
    r = run_bass_kernel_spmd(nc, in_maps, core_ids=list(range(NCORE)),
                             trace=TRACE)
    if getattr(r, "exec_time_ns", None):
        _tns[0] += r.exec_time_ns
    return r.results


class _Q:
    """Ordered DMA scheduler for one HWDGE ring. k rotating semaphores,
    <=k transfers in flight, so per-parity counts identify transfers."""

    def __init__(self, eng, *sems):
        self.eng, self.sems, self.n = eng, tuple(sems), 0
        self.k = len(self.sems)

    def dma(self, out, in_):
        idx = self.n
        self.n += 1
        p = idx % self.k
        if idx >= self.k:
            self.eng.wait_ge(self.sems[p], 16 * (idx // self.k))
        self.eng.dma_start(out=out, in_=in_).then_inc(self.sems[p], 16)
        return (self.sems[p], 16 * (idx // self.k + 1))

    def barrier_vals(self):
        return [(self.sems[p], 16 * ((self.n - p + self.k - 1) // self.k))
                for p in range(self.k)]


def _ngroups(nbytes):
    return max(1, min(8, int(nbytes // (1 << 18))))


# -------------------------------------------------------------------------
# host-side chunk layouts (all outputs contiguous -> efficient DMA lines)
# -------------------------------------------------------------------------
def _chunk_pairs(M, gnum, np_dt):
    """[n, w] -> [gnum, 128, 2, G, w] DoubleRow pair layout."""
    n, w = M.shape
    nch2 = n // 256
    G = nch2 // gnum
    a = M.reshape(gnum, G, 2, 128, w).transpose(0, 3, 2, 1, 4)
    return np.ascontiguousarray(a.astype(np_dt))


def _chunk_plain(M, gnum, np_dt):
    """[n, w] -> [gnum, 128, G, w] single-row layout."""
    n, w = M.shape
    nch = n // 128
    G = nch // gnum
    a = M.reshape(gnum, G, 128, w).transpose(0, 2, 1, 3)
    return np.ascontiguousarray(a.astype(np_dt))


def _pack_hilo(v):
    """[n, 16] f32 -> [128, nch, 32] f16 ([hi | lo] along the last axis)."""
    n = v.shape[0]
    nch = n // 128
    vh = v.astype(f16)
    vl = (v - vh.astype(np.float32)).astype(f16)
    a = np.concatenate([vh, vl], axis=1).reshape(nch, 128, 2 * D)
    return np.ascontiguousarray(a.transpose(1, 0, 2))


def _chunk_f32(v):
    """[n, 16] f32 -> [128, nch, 16] f32."""
    n = v.shape[0]
    nch = n // 128
    return np.ascontiguousarray(v.reshape(nch, 128, D).transpose(1, 0, 2))


# -------------------------------------------------------------------------
# GCN aggregation NEFF: agg[32, wg] = [vh|vl]^T @ at  (at = A^T col-slice)
# dr8: both operands fp8, DoubleRow (2x128 contraction/instr) — up path only.
# -------------------------------------------------------------------------
def build_gcn(n, at8, dr8=False):
    key = ("gcn", n, at8, dr8)
    if key in _cache:
        return _cache[key]
    wg = n // NCORE
    nch = n // 256 if dr8 else n // 128
    dt_at = DT8 if at8 else DT16
    dt_v = DT8 if dr8 else DT16
    esz = 1 if at8 else 2
    ga = _ngroups(n * wg * esz)
    Ga = nch // ga

    nc = bass.Bass(num_devices=NCORE)
    if dr8:
        at = nc.declare_dram_parameter("at", [ga, 128, 2, Ga, wg], dt_at,
                                       isOutput=False)
        v2 = nc.declare_dram_parameter("v2", [128, 2, nch, 2 * D], dt_v,
                                       isOutput=False)
    else:
        at = nc.declare_dram_parameter("at", [ga, 128, Ga, wg], dt_at,
                                       isOutput=False)
        v2 = nc.declare_dram_parameter("v2", [128, nch, 2 * D], dt_v,
                                       isOutput=False)
    agg = nc.declare_dram_parameter("agg", [2 * D, wg], DT32, isOutput=True)

    import contextlib
    with contextlib.ExitStack() as ctx:
        if dr8:
            at_sb = ctx.enter_context(
                nc.sbuf_tensor("at_sb", [128, 2, nch, wg], dt_at))
            v2_sb = ctx.enter_context(
                nc.sbuf_tensor("v2_sb", [128, 2, nch, 2 * D], dt_v))
        else:
            at_sb = ctx.enter_context(
                nc.sbuf_tensor("at_sb", [128, nch, wg], dt_at))
            v2_sb = ctx.enter_context(
                nc.sbuf_tensor("v2_sb", [128, nch, 2 * D], dt_v))
        ag_sb = ctx.enter_context(nc.sbuf_tensor("ag_sb", [2 * D, wg], DT32))
        ps = ctx.enter_context(nc.psum_tensor("ps", [128, 512], DT32))
        s_s1 = ctx.enter_context(nc.semaphore("s_s1"))
        s_s2 = ctx.enter_context(nc.semaphore("s_s2"))
        s_s3 = ctx.enter_context(nc.semaphore("s_s3"))
        s_a1 = ctx.enter_context(nc.semaphore("s_a1"))
        s_a2 = ctx.enter_context(nc.semaphore("s_a2"))
        s_a3 = ctx.enter_context(nc.semaphore("s_a3"))
        s_o = ctx.enter_context(nc.semaphore("s_o"))
        mm = ctx.enter_context(nc.semaphore("mm"))
        vec = ctx.enter_context(nc.semaphore("vec"))
        block = ctx.enter_context(nc.Block(no_gpsimd_drain=True))

        recs = {}

        @block.sync
        def _(sync):
            qs = _Q(sync, s_s1, s_s2, s_s3)
            for g in range(0, ga, 2):
                dst = (at_sb[:, :, g * Ga:(g + 1) * Ga, :] if dr8
                       else at_sb[:, g * Ga:(g + 1) * Ga, :])
                recs["g", g] = qs.dma(dst, at[g])
            sync.wait_ge(vec, 1)
            sync.dma_start(out=agg[:], in_=ag_sb[:]).then_inc(s_o, 16)
            sync.wait_ge(s_o, 16)
            for sem, v in qs.barrier_vals():
                sync.wait_ge(sem, v)

        @block.scalar
        def _(scalar):
            qa = _Q(scalar, s_a1, s_a2, s_a3)
            recs["v2"] = qa.dma(v2_sb[:], v2[:])
            for g in range(1, ga, 2):
                dst = (at_sb[:, :, g * Ga:(g + 1) * Ga, :] if dr8
                       else at_sb[:, g * Ga:(g + 1) * Ga, :])
                recs["g", g] = qa.dma(dst, at[g])
            for sem, v in qa.barrier_vals():
                scalar.wait_ge(sem, v)

        @block.tensor
        def _(tensor):
            tensor.wait_ge(*recs["v2"])
            ins = None
            for g in range(ga):
                tensor.wait_ge(*recs["g", g])
                for i in range(g * Ga, (g + 1) * Ga):
                    if dr8:
                        ins = nc.tensor.matmul(
                            ps[0:2 * D, 0:wg], lhsT=v2_sb[:, :, i, :],
                            rhs=at_sb[:, :, i, :],
                            start=(i == 0), stop=(i == nch - 1),
                            perf_mode=mybir.MatmulPerfMode.DoubleRow)
                    else:
                        ins = nc.tensor.matmul(
                            ps[0:2 * D, 0:wg], lhsT=v2_sb[:, i, :],
                            rhs=at_sb[:, i, :],
                            start=(i == 0), stop=(i == nch - 1))
            ins.then_inc(mm, 1)

        @block.vector
        def _(vector):
            vector.wait_ge(mm, 1)
            nc.vector.tensor_copy(ag_sb[:], ps[0:2 * D, 0:wg]).then_inc(vec, 1)

    _cache[key] = nc
    return nc


LO8 = 256.0  # keeps the fp8 lo plane out of subnormal range


def _pack_hilo8(v):
    """[n, 16] f32 -> [128, 2, nch2, 32] fp8 DoubleRow pair layout of the
    [hi | lo*256] fp8 split (up-path accuracy: ~1e-5 relative)."""
    n = v.shape[0]
    nch2 = n // 256
    vh = v.astype(f8)
    vl = ((v - vh.astype(np.float32)) * LO8).astype(f8)
    a = np.concatenate([vh.astype(np.float32), vl.astype(np.float32)],
                       axis=1).reshape(nch2, 2, 128, 2 * D)
    return np.ascontiguousarray(a.transpose(2, 1, 0, 3).astype(f8))


def _gcn_launch(n, at_chunks, v, dr8=False):
    """at_chunks: list of NCORE pre-chunked arrays.
    v [n,16] f32. Returns raw aggregation [n,16] f32."""
    at8 = at_chunks[0].dtype == f8
    nc = build_gcn(n, at8, dr8)
    v2 = _pack_hilo8(v) if dr8 else _pack_hilo(v)
    in_maps = [{"at": at_chunks[c], "v2": v2} for c in range(NCORE)]
    outs = _run(nc, in_maps)
    wg = n // NCORE
    losc = 1.0 / LO8 if dr8 else 1.0
    P = np.empty((n, D), np.float32)
    for c, o in enumerate(outs):
        a = o["agg"]
        P[c * wg:(c + 1) * wg] = (a[0:D] + a[D:2 * D] * losc).T
    return P


# -------------------------------------------------------------------------
# Level NEFF (n -> k pooled): CT block = B^T @ RTcols, then
# agg[32, kr] = [vh|vl]^T @ CT  (contraction over this core's col-block).
#   in8: fp8 DoubleRow C matmul; else fp16 plain.
#   ct_kind: "f8" | "f16" (emitted) | "f32r" (kept in SBUF only)
# -------------------------------------------------------------------------
def build_level(n, k, in8, ct_kind):
    key = ("lvl", n, k, in8, ct_kind)
    if key in _cache:
        return _cache[key]
    kr, kc = k // RG, k // CG
    icn = kc // 128
    grn = (kr + 511) // 512
    nd = icn * grn                    # number of CT psum drains
    if in8:
        nch2 = n // 256
        esz = 1
    else:
        nch2 = n // 128               # plain chunks
        esz = 2
    gr_n = _ngroups(n * kr * esz)
    gb_n = _ngroups(n * kc * esz)
    gnum = max(gr_n, gb_n)
    G = nch2 // gnum
    tck = icn
    dt_in = DT8 if in8 else DT16
    dt_ct = {"f8": DT8, "f16": DT16, "f32r": DT32R}[ct_kind]
    dt_v = DT16 if ct_kind != "f32r" else DT32R
    vw = 2 * D if ct_kind != "f32r" else D
    emit_ct = ct_kind != "f32r"

    nc = bass.Bass(num_devices=NCORE)
    if in8:
        rT = nc.declare_dram_parameter("rT", [gnum, 128, 2, G, kr], dt_in,
                                       isOutput=False)
        bT = nc.declare_dram_parameter("bT", [gnum, 128, 2, G, kc], dt_in,
                                       isOutput=False)
    else:
        rT = nc.declare_dram_parameter("rT", [gnum, 128, G, kr], dt_in,
                                       isOutput=False)
        bT = nc.declare_dram_parameter("bT", [gnum, 128, G, kc], dt_in,
                                       isOutput=False)
    v2 = nc.declare_dram_parameter("v2", [128, tck, vw], dt_v, isOutput=False)
    if emit_ct:
        ct = nc.declare_dram_parameter("ct", [128, icn, kr], dt_ct,
                                       isOutput=True)
    agg = nc.declare_dram_parameter("agg", [vw, kr], DT32, isOutput=True)

    import contextlib
    with contextlib.ExitStack() as ctx:
        if in8:
            r_sb = ctx.enter_context(
                nc.sbuf_tensor("r_sb", [128, 2, nch2, kr], dt_in))
            b_sb = ctx.enter_context(
                nc.sbuf_tensor("b_sb", [128, 2, nch2, kc], dt_in))
        else:
            r_sb = ctx.enter_context(
                nc.sbuf_tensor("r_sb", [128, nch2, kr], dt_in))
            b_sb = ctx.enter_context(
                nc.sbuf_tensor("b_sb", [128, nch2, kc], dt_in))
        v2_sb = ctx.enter_context(nc.sbuf_tensor("v2_sb", [128, tck, vw], dt_v))
        ct_sb = ctx.enter_context(nc.sbuf_tensor("ct_sb", [128, icn, kr], dt_ct))
        ag_sb = ctx.enter_context(nc.sbuf_tensor("ag_sb", [vw, kr], DT32))
        # the aggregation reuses banks pc[0..grn-1] after their CT drain
        pc = [ctx.enter_context(nc.psum_tensor(f"pc{i}", [128, 512], DT32))
              for i in range(nd)]
        s_s1 = ctx.enter_context(nc.semaphore("s_s1"))
        s_s2 = ctx.enter_context(nc.semaphore("s_s2"))
        s_s3 = ctx.enter_context(nc.semaphore("s_s3"))
        s_a1 = ctx.enter_context(nc.semaphore("s_a1"))
        s_a2 = ctx.enter_context(nc.semaphore("s_a2"))
        s_a3 = ctx.enter_context(nc.semaphore("s_a3"))
        s_oc = ctx.enter_context(nc.semaphore("s_oc"))
        s_oa = ctx.enter_context(nc.semaphore("s_oa"))
        mm = ctx.enter_context(nc.semaphore("mm"))
        vec = ctx.enter_context(nc.semaphore("vec"))
        block = ctx.enter_context(nc.Block(no_gpsimd_drain=True))

        recs = {}

        def _rs(g):
            return (r_sb[:, :, g * G:(g + 1) * G, :] if in8
                    else r_sb[:, g * G:(g + 1) * G, :])

        def _bs(g):
            return (b_sb[:, :, g * G:(g + 1) * G, :] if in8
                    else b_sb[:, g * G:(g + 1) * G, :])

        @block.sync
        def _(sync):
            qs = _Q(sync, s_s1, s_s2, s_s3)
            for g in range(gnum):
                if g % 2 == 0:
                    recs["r", g] = qs.dma(_rs(g), rT[g])
                else:
                    recs["b", g] = qs.dma(_bs(g), bT[g])
            if emit_ct:
                for ic in range(icn):
                    sync.wait_ge(vec, (ic + 1) * grn)
                    sync.dma_start(out=ct[:, ic, :], in_=ct_sb[:, ic, :]
                                   ).then_inc(s_oc, 16)
                sync.wait_ge(s_oc, 16 * icn)
            for sem, v in qs.barrier_vals():
                sync.wait_ge(sem, v)

        @block.scalar
        def _(scalar):
            qa = _Q(scalar, s_a1, s_a2, s_a3)
            for g in range(gnum):
                if g % 2 == 0:
                    recs["b", g] = qa.dma(_bs(g), bT[g])
                else:
                    recs["r", g] = qa.dma(_rs(g), rT[g])
            recs["v2"] = qa.dma(v2_sb[:], v2[:])
            scalar.wait_ge(vec, nd + 1)
            scalar.dma_start(out=agg[:], in_=ag_sb[:]).then_inc(s_oa, 16)
            scalar.wait_ge(s_oa, 16)
            for sem, v in qa.barrier_vals():
                scalar.wait_ge(sem, v)

        # the C matmul is emitted in two output-halves so the first half's
        # psum drains, ct DMA-out and aggregation head overlap the second
        # half's compute
        nh = 2 if icn >= 2 else 1
        hs = icn // nh

        @block.tensor
        def _(tensor):
            ins = None
            for h in range(nh):
                for g in range(gnum):
                    tensor.wait_ge(*recs["r", g])
                    tensor.wait_ge(*recs["b", g])
                    for i in range(g * G, (g + 1) * G):
                        for ic in range(h * hs, (h + 1) * hs):
                            for gr in range(grn):
                                w = min(512, kr - gr * 512)
                                if in8:
                                    ins = nc.tensor.matmul(
                                        pc[ic * grn + gr][0:128, 0:w],
                                        lhsT=b_sb[:, :, i,
                                                  ic * 128:(ic + 1) * 128],
                                        rhs=r_sb[:, :, i,
                                                 gr * 512:gr * 512 + w],
                                        start=(i == 0), stop=(i == nch2 - 1),
                                        perf_mode=mybir.MatmulPerfMode.DoubleRow)
                                else:
                                    ins = nc.tensor.matmul(
                                        pc[ic * grn + gr][0:128, 0:w],
                                        lhsT=b_sb[:, i,
                                                  ic * 128:(ic + 1) * 128],
                                        rhs=r_sb[:, i, gr * 512:gr * 512 + w],
                                        start=(i == 0), stop=(i == nch2 - 1))
                ins.then_inc(mm, 1)
            tensor.wait_ge(*recs["v2"])
            for tc in range(tck):
                tensor.wait_ge(vec, (tc + 1) * grn)
                for gr in range(grn):
                    w = min(512, kr - gr * 512)
                    ins = nc.tensor.matmul(
                        pc[gr][0:vw, 0:w], lhsT=v2_sb[:, tc, :],
                        rhs=ct_sb[:, tc, gr * 512:gr * 512 + w],
                        start=(tc == 0), stop=(tc == tck - 1),
                        skip_group_check=True)
            ins.then_inc(mm, 1)

        @block.vector
        def _(vector):
            for h in range(nh):
                vector.wait_ge(mm, h + 1)
                for tc in range(h * hs, (h + 1) * hs):
                    for gr in range(grn):
                        w = min(512, kr - gr * 512)
                        nc.vector.tensor_copy(
                            ct_sb[:, tc, gr * 512:gr * 512 + w],
                            pc[tc * grn + gr][0:128, 0:w]).then_inc(vec, 1)
            vector.wait_ge(mm, nh + 1)
            ins = None
            for gr in range(grn):
                w = min(512, kr - gr * 512)
                ins = nc.vector.tensor_copy(ag_sb[0:vw, gr * 512:gr * 512 + w],
                                            pc[gr][0:vw, 0:w])
            ins.then_inc(vec, 1)

    _cache[key] = nc
    return nc


def _level_launch(n, k, Ap, ApT, keep, z, dis, in8, ct_kind):
    """Ap/ApT f32 [n,n] = A' (unit-ish diag, small ints), keep sorted [k],
    z [k,16] f32 pooled features @ W, dis [k] f32 (host-computed).
    Returns (C f32 [k,k] w/ true diag | None, P [k,16] raw aggregation)."""
    nc = build_level(n, k, in8, ct_kind)
    kr, kc = k // RG, k // CG
    icn = kc // 128
    np_in = f8 if in8 else f16
    chunk = _chunk_pairs if in8 else _chunk_plain
    esz = 1 if in8 else 2
    gnum = max(_ngroups(n * kr * esz), _ngroups(n * kc * esz))

    v = dis[:, None] * z
    emit_ct = ct_kind != "f32r"

    in_maps = []
    for c in range(NCORE):
        r, cc = divmod(c, CG)
        kwr = keep[r * kr:(r + 1) * kr]
        kwc = keep[cc * kc:(cc + 1) * kc]
        vs = v[cc * kc:(cc + 1) * kc]
        if ct_kind == "f32r":
            v2 = _chunk_f32(vs)
        else:
            v2 = _pack_hilo(vs)
        in_maps.append({
            "rT": chunk(ApT[:, kwr], gnum, np_in),
            "bT": chunk(Ap[:, kwc], gnum, np_in),
            "v2": v2,
        })
    outs = _run(nc, in_maps)

    vw = 2 * D if ct_kind != "f32r" else D
    P = np.zeros((k, D), np.float32)
    C = np.empty((k, k), np.float32) if emit_ct else None
    for c, o in enumerate(outs):
        r, cc = divmod(c, CG)
        a = o["agg"]
        blk = (a[0:D] + a[D:2 * D]) if vw == 2 * D else a
        P[r * kr:(r + 1) * kr] += blk.T
        if emit_ct:
            cblk = o["ct"].astype(np.float32)  # [128, icn, kr]
            cblk = cblk.transpose(1, 0, 2).reshape(kc, kr)
            C[r * kr:(r + 1) * kr, cc * kc:(cc + 1) * kc] = cblk.T
    return C, P


def _host_deg(Ap, ApT, keep):
    big = Ap[:, keep]
    colsum = big.sum(axis=1, dtype=np.float64)
    rows = Ap[keep, :]
    colsT = ApT[keep, :]
    rowsumC = rows.astype(np.float64) @ colsum
    diagC = np.einsum("ij,ij->i", rows, colsT)
    deg = rowsumC - diagC + 1.0
    dis = (1.0 / np.sqrt(deg)).astype(np.float32)
    return dis, diagC.astype(np.float32)


def _topk(x, p):
    s = np.tanh((x @ (p / np.linalg.norm(p))).astype(np.float64))
    k = x.shape[0] // 2
    order = np.argsort(-s, kind="stable")
    keep = np.sort(order[:k])
    return keep, s[keep].astype(np.float32)


def _at_slices(AT, np_dt, dr8=False):
    """Per-core chunked column-slices of AT [n, n] for the gcn NEFF."""
    n = AT.shape[0]
    wg = n // NCORE
    ga = _ngroups(n * wg * (1 if np_dt == f8 else 2))
    chunk = _chunk_pairs if dr8 else _chunk_plain
    return [chunk(AT[:, c * wg:(c + 1) * wg], ga, np_dt)
            for c in range(NCORE)]


def kernel(**inputs):
    x = np.asarray(inputs["x"], np.float32)
    ei = np.asarray(inputs["edge_index"]).astype(np.int64)
    W = {kk: np.asarray(v, np.float32) for kk, v in inputs.items()
         if kk not in ("x", "edge_index")}

    # dense adjacency
    A = np.zeros((N, N), np.float32)
    np.add.at(A, (ei[0], ei[1]), 1.0)
    d0 = np.diagonal(A).copy()
    fix = (d0 == 0).astype(np.float32)
    deg0 = A.sum(1) + fix
    dis0 = 1.0 / np.sqrt(deg0)
    Ag0 = A.copy()
    np.fill_diagonal(Ag0, d0 + fix)
    assert Ag0.max() <= 15, "adjacency exceeds fp8-exact range"
    Ag0T = np.ascontiguousarray(Ag0.T)
    at0_8 = _at_slices(Ag0T, f8)
    Ap0 = A
    np.fill_diagonal(Ap0, 1.0)
    ApT0 = np.ascontiguousarray(Ap0.T)

    # GCN0
    v0 = dis0[:, None] * (x @ W["W_d0"])
    P0 = _gcn_launch(N, at0_8, v0)
    x0 = np.maximum(dis0[:, None] * P0 + W["b_d0"], 0.0)

    # ---- down path ----
    keep0, vals0 = _topk(x0, W["p0"])
    z1 = (x0[keep0] * vals0[:, None]) @ W["W_d1"]
    dis1, diagC0 = _host_deg(Ap0, ApT0, keep0)
    C0, P1 = _level_launch(N, KS[0], Ap0, ApT0, keep0, z1, dis1, True, "f8")
    v1 = dis1[:, None] * z1
    P1 += (1.0 - diagC0)[:, None] * v1
    x1 = np.maximum(dis1[:, None] * P1 + W["b_d1"], 0.0)
    A1 = C0
    np.fill_diagonal(A1, 1.0)
    assert A1.max() <= 15, "level-1 adjacency exceeds fp8-exact range"
    A1T = np.ascontiguousarray(A1.T)

    keep1, vals1 = _topk(x1, W["p1"])
    z2 = (x1[keep1] * vals1[:, None]) @ W["W_d2"]
    dis2, diagC1 = _host_deg(A1, A1T, keep1)
    C1, P2 = _level_launch(KS[0], KS[1], A1, A1T, keep1, z2, dis2, True, "f16")
    v2_ = dis2[:, None] * z2
    P2 += (1.0 - diagC1)[:, None] * v2_
    x2 = np.maximum(dis2[:, None] * P2 + W["b_d2"], 0.0)
    A2 = C1
    np.fill_diagonal(A2, 1.0)
    assert A2.max() <= 2048, "level-2 adjacency exceeds fp16-exact range"
    A2T = np.ascontiguousarray(A2.T)

    keep2, vals2 = _topk(x2, W["p2"])
    z3 = (x2[keep2] * vals2[:, None]) @ W["W_d3"]
    dis3, diagC2 = _host_deg(A2, A2T, keep2)
    _, P3 = _level_launch(KS[1], KS[2], A2, A2T, keep2, z3, dis3, False,
                          "f32r")
    v3 = dis3[:, None] * z3
    P3 += (1.0 - diagC2)[:, None] * v3
    x3 = np.maximum(dis3[:, None] * P3 + W["b_d3"], 0.0)

    # ---- up path ----
    xin = x2.copy()
    xin[keep2] += x3
    Pu = _gcn_launch(KS[1], _at_slices(A2T, f16),
                     dis2[:, None] * (xin @ W["W_u0"]))
    xu = np.maximum(dis2[:, None] * Pu + W["b_u0"], 0.0)

    xin = x1.copy()
    xin[keep1] += xu
    Pu = _gcn_launch(KS[0], _at_slices(A1T, f8),
                     dis1[:, None] * (xin @ W["W_u1"]))
    xu = np.maximum(dis1[:, None] * Pu + W["b_u1"], 0.0)

    xin = x0.copy()
    xin[keep0] += xu
    Pu = _gcn_launch(N, _at_slices(Ag0T, f8, dr8=True),
                     dis0[:, None] * (xin @ W["W_u2"]), dr8=True)
    return (dis0[:, None] * Pu + W["b_u2"]).astype(np.float32)
